# revision 2
# baseline (speedup 1.0000x reference)
"""CARAFE-naive 2x content-aware upsampling on 8 Trainium2 NeuronCores.

Problem: features [2, 256, 100, 100] f32, masks [2, 25, 200, 200] f32
-> out [2, 256, 200, 200] f32, where each output pixel is a 25-tap (5x5)
weighted sum of the source neighborhood, weights shared across channels.

Strategy (per core = one (image n, row-quarter q) pair), "v4 2D-patch":
  The full 25-tap contraction of every output pixel is done by a SINGLE
  TensorE matmul column. The stationary operand (lhsT) is a 9x14 patch
  of padded features laid out on K = 9*14 = 126 partitions (padded to
  128), M = 128 channels. One patch covers the complete 5x5 neighborhoods
  of a 5 (rows) x 10 (cols) block of low-res pixels, i.e. N = 5*2*10*2 =
  200 output columns per matmul. The moving operand (rhs) is a host-
  packed mask tensor whose column (dh, a, wl, b) holds the pixel's 25
  mask values scattered to the K rows of its taps (zeros elsewhere).

  Per core: 5 h-bands x 10 w-tiles x 2 channel-halves = 100 independent
  single-shot matmuls (start=stop=True, no PSUM accumulation chains).
  PSUM tiles own a full 2KB bank each. PSUM->SBUF copies (f32 -> f16,
  alternating Vector/Scalar engines) assemble a [128, 10, 200] output
  band which is DMA'd to DRAM in output layout. Inputs ride the sync
  HWDGE ring, outputs the scalar ring. Output is fp16 on device and
  widened to f32 on the host (inputs are likewise fp16-cast on host).

Host-side numpy does layout/packing only (transpose, pad, scatter of
mask values into patch-row positions); all FLOPs run on the device.
"""

import numpy as np

import concourse.mybir as mybir
import concourse.tile as tile
from concourse import bacc
from concourse.bass_utils import run_bass_kernel_spmd

# problem constants
N, C, H, W = 2, 256, 100, 100
KS = 5        # kernel size
S = 2         # upsample scale
R = (KS - 1) // 2

# sharding / blocking constants (v4)
HC = H // 4        # 25 low-res rows per core (8 cores = 2 images x 4 quarters)
NR = HC + 2 * R    # 29 padded feature rows per core
RH, RW = 9, 14     # patch rows x cols on partitions: K = 126 (<= 128)
BH = RH - (KS - 1)   # 5 low-res rows served per patch
BW = RW - (KS - 1)   # 10 low-res cols served per patch
NT = HC // BH        # 5 h-bands
NU = W // BW         # 10 w-tiles
NTILE = NT * NU      # 50 patch tiles per core
NCOL = BH * S * BW * S  # 200 matmul columns per tile: (dh, a, wl, b)
F16 = mybir.dt.float16
F32 = mybir.dt.float32


def build_program(iters: int = 1, parts: str = "full", psbufs: int = 6,
                  obufs: int = 3, in_chunks: int = 5):
    """Build the per-core bass program. `iters`>1 wraps the whole compute in
    a hardware loop (used only for benchmarking slope timing)."""
    nc = bacc.Bacc(None, target_bir_lowering=False, debug=False)
    f_in = nc.dram_tensor("f", [128, NTILE, C], F16, kind="ExternalInput")
    b_in = nc.dram_tensor("b", [128, NTILE, NCOL], F16, kind="ExternalInput")
    out = nc.dram_tensor("out", [C, S * HC, S * W], F16, kind="ExternalOutput")

    with tile.TileContext(nc) as tc:
        with (
            tc.tile_pool(name="fsb", bufs=1) as fpool,
            tc.tile_pool(name="bsb", bufs=1) as bpool,
            tc.tile_pool(name="osb", bufs=obufs) as opool,
            tc.tile_pool(name="ps", bufs=psbufs, space="PSUM") as pspool,
        ):
            def body(_=None):
                F_sb = fpool.tile([128, NTILE, C], F16)
                B_sb = bpool.tile([128, NTILE, NCOL], F16)
                if parts != "nodmain":
                    # chunked input DMAs, band-major so band t's matmuls can
                    # start as soon as chunk t has landed
                    step = (NTILE + in_chunks - 1) // in_chunks
                    for i0 in range(0, NTILE, step):
                        i1 = min(i0 + step, NTILE)
                        nc.sync.dma_start(F_sb[:, i0:i1], f_in[:, i0:i1])
                        nc.sync.dma_start(B_sb[:, i0:i1], b_in[:, i0:i1])
                if parts == "dmain":
                    return
                for t in range(NT):
                    osbs = [
                        opool.tile([128, BH, S, W, S], F16, name=f"o{t}_{ct}")
                        for ct in range(2)
                    ]
                    for u in range(NU):
                        idx = t * NU + u
                        for ct in range(2):
                            ps = pspool.tile(
                                [128, 512], F32, name=f"ps{t}_{u}_{ct}",
                                tag="ps",
                            )
                            nc.tensor.matmul(
                                ps[:, :NCOL],
                                F_sb[:, idx, ct * 128 : (ct + 1) * 128],
                                B_sb[:, idx, :],
                                start=True,
                                stop=True,
                            )
                            if parts == "nocopy":
                                continue
                            src = ps[:, :NCOL].rearrange(
                                "p (d a w b) -> p d a w b",
                                d=BH, a=S, w=BW, b=S,
                            )
                            dst = osbs[ct][:, :, :, BW * u : BW * (u + 1), :]
                            if (u + ct) % 2 == 0:
                                nc.vector.tensor_copy(dst, src)
                            else:
                                nc.scalar.copy(dst, src)
                    if parts in ("nocopy", "noout"):
                        continue
                    for ct in range(2):
                        nc.scalar.dma_start(
                            out[ct * 128 : (ct + 1) * 128,
                                S * BH * t : S * BH * (t + 1), :],
                            osbs[ct][:].rearrange("p d a w b -> p (d a) (w b)"),
                        )

            if iters == 1:
                body()
            else:
                with tc.For_i(0, iters, 1) as _i:
                    body(_i)
    nc.finalize()
    return nc


def host_prep_v2(features: np.ndarray, masks: np.ndarray):
    """v4 layouts (name kept for the test harness API).

    f: [128, 50, 256] fp16 -- partition k = pr*14+pc indexes the 9x14
       feature patch of tile (t, u); free dims (tile, channel).
    b: [128, 50, 200] fp16 -- partition k as above; column (dh, a, wl, b)
       holds mask[dy=pr-dh, dx=pc-wl] of pixel (h=5t+dh, w=10u+wl),
       subpixel (a, b); zero where (dy, dx) is outside the 5x5 window.
    """
    f_hosts, b_hosts = [], []
    padded = np.pad(features, ((0, 0), (0, 0), (R, R), (R, R)))  # [N,C,H+4,W+4]

    # index helpers for the banded scatter (shared across cores)
    dy = np.arange(KS)[:, None, None, None]        # [5,1,1,1]
    dx = np.arange(KS)[None, :, None, None]        # [1,5,1,1]
    dh = np.arange(BH)[None, None, :, None]        # [1,1,5,1]
    wl = np.arange(BW)[None, None, None, :]        # [1,1,1,10]
    kf = (dh + dy) * RW + (wl + dx)                # [5,5,5,10] patch row index
    dhb = np.broadcast_to(dh, kf.shape)
    wlb = np.broadcast_to(wl, kf.shape)

    rows = (BH * np.arange(NT)[:, None] + np.arange(RH)[None, :])  # [5,9]
    cols = (BW * np.arange(NU)[:, None] + np.arange(RW)[None, :])  # [10,14]

    for core in range(8):
        n, q = divmod(core, 4)
        h0 = HC * q
        # features: [C, 29, 104] padded window for this core
        F_core = padded[n, :, h0 : h0 + NR, :]
        P2 = F_core.transpose(1, 2, 0)  # [29, 104, C]
        # W5[t, pr, u, pc, c]
        W5 = P2[rows[:, :, None, None], cols[None, None, :, :], :]
        F_host = np.zeros((128, NTILE, C), np.float16)
        F_host[: RH * RW] = (
            W5.transpose(1, 3, 0, 2, 4).reshape(RH * RW, NTILE, C)
        )
        # masks: m7[dy, dx, h, a, w, b] for this core's rows
        m7 = masks[n].reshape(KS, KS, H, S, W, S)[:, :, h0 : h0 + HC]
        # -> [dy, dx, t, dh, a, u, wl, b] -> [dy, dx, dh, wl, t, u, a, b]
        m8 = m7.reshape(KS, KS, NT, BH, S, NU, BW, S)
        m8 = m8.transpose(0, 1, 3, 6, 2, 5, 4, 7)
        B8 = np.zeros((128, NT, NU, BH, S, BW, S), np.float16)
        # advanced indexing on (k, dh, wl) axes with [5,5,5,10] index arrays;
        # result axes order: [dy/dx/dh/wl bcast..., t, u, a, b]
        B8[kf, :, :, dhb, :, wlb, :] = m8
        b_hosts.append(
            B8.transpose(0, 1, 2, 3, 4, 5, 6).reshape(128, NTILE, NCOL)
        )
        f_hosts.append(F_host)
    return f_hosts, b_hosts


_NC_CACHE = {}


def _get_program(iters: int = 1):
    if iters not in _NC_CACHE:
        _NC_CACHE[iters] = build_program(iters)
    return _NC_CACHE[iters]


def kernel(features: np.ndarray, masks: np.ndarray) -> np.ndarray:
    features = np.ascontiguousarray(features, dtype=np.float32)
    masks = np.ascontiguousarray(masks, dtype=np.float32)
    f_hosts, b_hosts = host_prep_v2(features, masks)
    in_maps = [{"f": f_hosts[c], "b": b_hosts[c]} for c in range(8)]
    nc = _get_program(1)
    res = run_bass_kernel_spmd(nc, in_maps, list(range(8)))
    out = np.empty((N, C, S * H, S * W), np.float32)
    for core in range(8):
        n, q = divmod(core, 4)
        out[n, :, S * HC * q : S * HC * (q + 1), :] = (
            res.results[core]["out"].astype(np.float32)
        )
    return out


# revision 14
# speedup vs baseline: 3.1133x; 3.1133x over previous
"""CARAFE-naive 2x content-aware upsampling on 8 Trainium2 NeuronCores.

Problem: features [2, 256, 100, 100] f32, masks [2, 25, 200, 200] f32
-> out [2, 256, 200, 200] f32, where each output pixel is a 25-tap (5x5)
weighted sum of the source neighborhood, weights shared across channels.

Strategy (per core = one (image n, row-quarter q) pair), "2D-patch":
  The full 25-tap contraction of every output pixel is done by a SINGLE
  TensorE matmul column. The stationary operand (lhsT) is a 9x14 patch
  of padded features laid out on K = 9*14 = 126 partitions (padded to
  128), M = 128 channels. One patch covers the complete 5x5 neighborhoods
  of a 5 (rows) x 10 (cols) block of low-res pixels, i.e. N = 5*2*10*2 =
  200 output columns per matmul. The moving operand (rhs) is a host-
  packed mask tensor whose column (dh, a, wl, b) holds the pixel's 25
  mask values scattered to the K rows of its taps (zeros elsewhere).

  Per core: 5 h-bands x 10 w-tiles x 2 channel-halves = 100 independent
  single-shot matmuls (start=stop=True, no PSUM accumulation chains).
  Each band's features+masks are packed into one [128, 10, 456] input
  chunk (one DMA per band; bands 0-2 ride the sync HWDGE ring, bands 3-4
  the scalar ring so transfer tails overlap). PSUM tiles span 2 banks
  (one per channel half); a single fused PSUM->SBUF copy per tile
  (f32 -> f16, alternating Vector/Scalar engines) assembles paired
  output bands that go out in 3 DMAs on the sync ring. Output is fp16 on
  device, widened to f32 on host (inputs are fp16-cast on host too).
  The benchmark loop body holds `unroll` iterations to amortize the
  For_i back-edge barrier (~3.4us of DMA idle per back-edge).

  The kernel is DMA-bound: steady state moves 10.96 MB/iteration
  (5.84 in + 5.12 out) at the ~350 GB/s per-core HBM roofline, with
  TensorE (~15us), the PSUM-drain copies (~15us/engine), and all input
  waits hidden under the transfer schedule.

Host-side numpy does layout/packing only (transpose, pad, scatter of
mask values into patch-row positions); all FLOPs run on the device.
"""

import numpy as np

import concourse.mybir as mybir
import concourse.tile as tile
from concourse import bacc
from concourse.bass_utils import run_bass_kernel_spmd

# problem constants
N, C, H, W = 2, 256, 100, 100
KS = 5        # kernel size
S = 2         # upsample scale
R = (KS - 1) // 2

# sharding / blocking constants
HC = H // 4        # 25 low-res rows per core (8 cores = 2 images x 4 quarters)
NR = HC + 2 * R    # 29 padded feature rows per core
RH, RW = 9, 14     # patch rows x cols on partitions: K = 126 (<= 128)
BH = RH - (KS - 1)   # 5 low-res rows served per patch
BW = RW - (KS - 1)   # 10 low-res cols served per patch
NT = HC // BH        # 5 h-bands
NU = W // BW         # 10 w-tiles
NTILE = NT * NU      # 50 patch tiles per core
NCOL = BH * S * BW * S  # 200 matmul columns per tile: (dh, a, wl, b)
FBW = C + NCOL       # 456 packed free width: [features 256 | masks 200]
NTF = 3              # bands in the "f" input tensor (rest in "b")
PAIRS = [(0, 1), (2, 3), (4,)]  # output band grouping
F16 = mybir.dt.float16
F32 = mybir.dt.float32


def build_program(iters: int = 1, parts: str = "full", psbufs: int = 4,
                  obufs: int = 2, unroll: int = 4):
    """Build the per-core bass program. `iters`>1 wraps the whole compute in
    a hardware loop (used only for benchmarking slope timing); the loop body
    holds `unroll` copies of the iteration to amortize the For_i back-edge
    barrier (each back-edge waits for every DMA completion of the body)."""
    nc = bacc.Bacc(None, target_bir_lowering=False, debug=False)
    f_in = nc.dram_tensor("f", [128, NTF * NU, FBW], F16, kind="ExternalInput")
    b_in = nc.dram_tensor("b", [128, (NT - NTF) * NU, FBW], F16,
                          kind="ExternalInput")
    out = nc.dram_tensor("out", [C, S * HC, S * W], F16, kind="ExternalOutput")

    with tile.TileContext(nc) as tc:
        with (
            tc.tile_pool(name="fbsb", bufs=1) as fbpool,
            tc.tile_pool(name="osb", bufs=obufs) as opool,
            tc.tile_pool(name="ps", bufs=psbufs, space="PSUM") as pspool,
        ):
            def body(_=None):
                # one packed [feat|mask] tile per band; band t's matmuls
                # start as soon as its chunk lands
                FB_ts = [
                    fbpool.tile([128, NU, FBW], F16, name=f"FB{t}",
                                tag=f"FB{t}")
                    for t in range(NT)
                ]
                if parts != "nodmain":
                    for t in range(NT):
                        if t < NTF:
                            nc.sync.dma_start(
                                FB_ts[t][:], f_in[:, t * NU : (t + 1) * NU]
                            )
                        else:
                            nc.scalar.dma_start(
                                FB_ts[t][:],
                                b_in[:, (t - NTF) * NU : (t - NTF + 1) * NU],
                            )
                if parts == "dmain":
                    return
                for pi, pair in enumerate(PAIRS):
                    pb = len(pair)
                    # [c, ct, band-in-pair, dh, a, w, b] staging
                    osb = opool.tile([128, 2, pb, BH, S, W, S], F16,
                                     name=f"o{pi}", tag=f"o{pi}")
                    for bi, t in enumerate(pair):
                        for u in range(NU):
                            # one PSUM tile = 2 banks; matmul ct writes bank ct
                            ps = pspool.tile(
                                [128, 2, 512], F32, name=f"ps{t}_{u}",
                                tag="ps",
                            )
                            for ct in range(2):
                                nc.tensor.matmul(
                                    ps[:, ct, :NCOL],
                                    FB_ts[t][:, u, ct * 128 : (ct + 1) * 128],
                                    FB_ts[t][:, u, C:],
                                    start=True,
                                    stop=True,
                                )
                            if parts == "nocopy":
                                continue
                            src = ps[:, :, :NCOL].rearrange(
                                "p c (d a w b) -> p c d a w b",
                                d=BH, a=S, w=BW, b=S,
                            )
                            dst = osb[:, :, bi, :, :, BW * u : BW * (u + 1), :]
                            if u % 2 == 0:
                                nc.vector.tensor_copy(dst, src)
                            else:
                                nc.scalar.copy(dst, src)
                    if parts in ("nocopy", "noout"):
                        continue
                    y0 = S * BH * pair[0]
                    nc.sync.dma_start(
                        out[:].rearrange(
                            "(c p) y x -> p c y x", c=2
                        )[:, :, y0 : y0 + S * BH * pb, :],
                        osb[:].rearrange("p c i d a w b -> p c (i d a) (w b)"),
                    )

            if iters <= unroll:
                for _k in range(iters):
                    body(_k)
            else:
                loop_n = iters // unroll
                rem = iters - loop_n * unroll
                with tc.For_i(0, loop_n, 1) as _i:
                    for _k in range(unroll):
                        body(_i)
                for _k in range(rem):
                    body(_k)
    nc.finalize()
    return nc


def host_prep_v2(features: np.ndarray, masks: np.ndarray):
    """Pack per-core fp16 inputs (name kept for the test harness API).

    Per band t (5 bands of 5 low-res rows), per w-tile u, a packed
    [128, 456] chunk: cols 0:256 the 9x14 feature patch (partition
    k = pr*14+pc), cols 256:456 the banded masks whose column
    (dh, a, wl, b) holds mask[dy=pr-dh, dx=pc-wl] of pixel
    (h=5t+dh, w=10u+wl), subpixel (a, b); zeros elsewhere.
    Bands 0-2 are returned as "f", bands 3-4 as "b".
    """
    f_hosts, b_hosts = [], []
    padded = np.pad(features, ((0, 0), (0, 0), (R, R), (R, R)))  # [N,C,H+4,W+4]

    # index helpers for the banded scatter (shared across cores)
    dy = np.arange(KS)[:, None, None, None]        # [5,1,1,1]
    dx = np.arange(KS)[None, :, None, None]        # [1,5,1,1]
    dh = np.arange(BH)[None, None, :, None]        # [1,1,5,1]
    wl = np.arange(BW)[None, None, None, :]        # [1,1,1,10]
    kf = (dh + dy) * RW + (wl + dx)                # [5,5,5,10] patch row index
    dhb = np.broadcast_to(dh, kf.shape)
    wlb = np.broadcast_to(wl, kf.shape)

    rows = (BH * np.arange(NT)[:, None] + np.arange(RH)[None, :])  # [5,9]
    cols = (BW * np.arange(NU)[:, None] + np.arange(RW)[None, :])  # [10,14]

    for core in range(8):
        n, q = divmod(core, 4)
        h0 = HC * q
        FB = np.zeros((128, NT, NU, FBW), np.float16)
        # features: [C, 29, 104] padded window for this core
        F_core = padded[n, :, h0 : h0 + NR, :]
        P2 = F_core.transpose(1, 2, 0)  # [29, 104, C]
        # W5[t, pr, u, pc, c] -> [pr*14+pc, t, u, c]
        W5 = P2[rows[:, :, None, None], cols[None, None, :, :], :]
        FB[: RH * RW, :, :, :C] = W5.transpose(1, 3, 0, 2, 4).reshape(
            RH * RW, NT, NU, C
        )
        # masks: m7[dy, dx, h, a, w, b] for this core's rows
        m7 = masks[n].reshape(KS, KS, H, S, W, S)[:, :, h0 : h0 + HC]
        # -> [dy, dx, dh, wl, t, u, a, b]
        m8 = m7.reshape(KS, KS, NT, BH, S, NU, BW, S)
        m8 = m8.transpose(0, 1, 3, 6, 2, 5, 4, 7)
        B8 = np.zeros((128, NT, NU, BH, S, BW, S), np.float16)
        B8[kf, :, :, dhb, :, wlb, :] = m8
        FB[:, :, :, C:] = B8.reshape(128, NT, NU, NCOL)
        f_hosts.append(
            np.ascontiguousarray(FB[:, :NTF].reshape(128, NTF * NU, FBW))
        )
        b_hosts.append(
            np.ascontiguousarray(FB[:, NTF:].reshape(128, (NT - NTF) * NU, FBW))
        )
    return f_hosts, b_hosts


_NC_CACHE = {}


def _get_program(iters: int = 1):
    if iters not in _NC_CACHE:
        _NC_CACHE[iters] = build_program(iters)
    return _NC_CACHE[iters]


def kernel(features: np.ndarray, masks: np.ndarray) -> np.ndarray:
    features = np.ascontiguousarray(features, dtype=np.float32)
    masks = np.ascontiguousarray(masks, dtype=np.float32)
    f_hosts, b_hosts = host_prep_v2(features, masks)
    in_maps = [{"f": f_hosts[c], "b": b_hosts[c]} for c in range(8)]
    nc = _get_program(1)
    res = run_bass_kernel_spmd(nc, in_maps, list(range(8)))
    out = np.empty((N, C, S * H, S * W), np.float32)
    for core in range(8):
        n, q = divmod(core, 4)
        out[n, :, S * HC * q : S * HC * (q + 1), :] = (
            res.results[core]["out"].astype(np.float32)
        )
    return out


# revision 17
# speedup vs baseline: 4.4088x; 1.4161x over previous
"""CARAFE-naive 2x content-aware upsampling on 8 Trainium2 NeuronCores.

Problem: features [2, 256, 100, 100] f32, masks [2, 25, 200, 200] f32
-> out [2, 256, 200, 200] f32, where each output pixel is a 25-tap (5x5)
weighted sum of the source neighborhood, weights shared across channels.

Strategy (per core = one (image n, row-quarter q) pair), "2D-patch":
  The full 25-tap contraction of every output pixel is done by a SINGLE
  TensorE matmul column. The stationary operand (lhsT) is a 9x14 patch
  of padded features laid out on K = 9*14 = 126 partitions (padded to
  128), M = 128 channels. One patch covers the complete 5x5 neighborhoods
  of a 5 (rows) x 10 (cols) block of low-res pixels, i.e. N = 5*2*10*2 =
  200 output columns per matmul. The moving operand (rhs) is a host-
  packed mask tensor whose column (dh, a, wl, b) holds the pixel's 25
  mask values scattered to the K rows of its taps (zeros elsewhere).

  Per core: 5 h-bands x 10 w-tiles x 2 channel-halves = 100 independent
  single-shot matmuls (start=stop=True, no PSUM accumulation chains).
  Each band's features+masks are packed into one [128, 10, 456] input
  chunk (one DMA per band; bands 0-2 ride the sync HWDGE ring, bands 3-4
  the scalar ring so transfer tails overlap). PSUM tiles span 2 banks
  (one per channel half); a single fused PSUM->SBUF copy per tile
  (f32 -> f16, alternating Vector/Scalar engines) assembles paired
  output bands that go out in 3 DMAs on the sync ring. Output is fp16 on
  device, widened to f32 on host (inputs are fp16-cast on host too).
  The benchmark loop body holds `unroll` iterations to amortize the
  For_i back-edge barrier (~3.4us of DMA idle per back-edge).

  The kernel is DMA-bound: steady state moves 10.96 MB/iteration
  (5.84 in + 5.12 out) at the ~350 GB/s per-core HBM roofline, with
  TensorE (~15us), the PSUM-drain copies (~15us/engine), and all input
  waits hidden under the transfer schedule.

Host-side numpy does layout/packing only (transpose, pad, scatter of
mask values into patch-row positions); all FLOPs run on the device.
"""

import numpy as np

import concourse.mybir as mybir
import concourse.tile as tile
from concourse import bacc
from concourse.bass_utils import run_bass_kernel_spmd

# problem constants
N, C, H, W = 2, 256, 100, 100
KS = 5        # kernel size
S = 2         # upsample scale
R = (KS - 1) // 2

# sharding / blocking constants
HC = H // 4        # 25 low-res rows per core (8 cores = 2 images x 4 quarters)
NR = HC + 2 * R    # 29 padded feature rows per core
RH, RW = 9, 14     # patch rows x cols on partitions: K = 126 (<= 128)
BH = RH - (KS - 1)   # 5 low-res rows served per patch
BW = RW - (KS - 1)   # 10 low-res cols served per patch
NT = HC // BH        # 5 h-bands
NU = W // BW         # 10 w-tiles
NTILE = NT * NU      # 50 patch tiles per core
NCOL = BH * S * BW * S  # 200 matmul columns per tile: (dh, a, wl, b)
FBW = C + NCOL       # 456 packed free width: [features 256 | masks 200]
NTF = 3              # bands in the "f" input tensor (rest in "b")
PAIRS = [(0, 1), (2, 3), (4,)]  # output band grouping
F16 = mybir.dt.float16
F32 = mybir.dt.float32


def build_program(iters: int = 1, parts: str = "full", psbufs: int = 4,
                  obufs: int = 2, unroll: int = 8):
    """Build the per-core bass program. `iters`>1 wraps the whole compute in
    a hardware loop (used only for benchmarking slope timing); the loop body
    holds `unroll` copies of the iteration to amortize the For_i back-edge
    barrier (each back-edge waits for every DMA completion of the body)."""
    nc = bacc.Bacc(None, target_bir_lowering=False, debug=False)
    f_in = nc.dram_tensor("f", [128, NTF * NU, FBW], F16, kind="ExternalInput")
    b_in = nc.dram_tensor("b", [128, (NT - NTF) * NU, FBW], F16,
                          kind="ExternalInput")
    out = nc.dram_tensor("out", [C, S * HC, S * W], F16, kind="ExternalOutput")

    with tile.TileContext(nc) as tc:
        with (
            tc.tile_pool(name="fbsb", bufs=1) as fbpool,
            tc.tile_pool(name="osb", bufs=obufs) as opool,
            tc.tile_pool(name="ps", bufs=psbufs, space="PSUM") as pspool,
        ):
            def body(_=None):
                # one packed [feat|mask] tile per band; band t's matmuls
                # start as soon as its chunk lands
                FB_ts = [
                    fbpool.tile([128, NU, FBW], F16, name=f"FB{t}",
                                tag=f"FB{t}")
                    for t in range(NT)
                ]
                if parts != "nodmain":
                    for t in range(NT):
                        if t < NTF:
                            nc.sync.dma_start(
                                FB_ts[t][:], f_in[:, t * NU : (t + 1) * NU]
                            )
                        else:
                            nc.scalar.dma_start(
                                FB_ts[t][:],
                                b_in[:, (t - NTF) * NU : (t - NTF + 1) * NU],
                            )
                if parts == "dmain":
                    return
                for pi, pair in enumerate(PAIRS):
                    pb = len(pair)
                    # [c, ct, band-in-pair, dh, a, w, b] staging
                    osb = opool.tile([128, 2, pb, BH, S, W, S], F16,
                                     name=f"o{pi}", tag=f"o{pi}")
                    for bi, t in enumerate(pair):
                        for u in range(NU):
                            # one PSUM tile = 2 banks; matmul ct writes bank ct
                            ps = pspool.tile(
                                [128, 2, 512], F32, name=f"ps{t}_{u}",
                                tag="ps",
                            )
                            for ct in range(2):
                                nc.tensor.matmul(
                                    ps[:, ct, :NCOL],
                                    FB_ts[t][:, u, ct * 128 : (ct + 1) * 128],
                                    FB_ts[t][:, u, C:],
                                    start=True,
                                    stop=True,
                                )
                            if parts == "nocopy":
                                continue
                            src = ps[:, :, :NCOL].rearrange(
                                "p c (d a w b) -> p c d a w b",
                                d=BH, a=S, w=BW, b=S,
                            )
                            dst = osb[:, :, bi, :, :, BW * u : BW * (u + 1), :]
                            if u % 2 == 0:
                                nc.vector.tensor_copy(dst, src)
                            else:
                                nc.scalar.copy(dst, src)
                    if parts in ("nocopy", "noout"):
                        continue
                    y0 = S * BH * pair[0]
                    nc.sync.dma_start(
                        out[:].rearrange(
                            "(c p) y x -> p c y x", c=2
                        )[:, :, y0 : y0 + S * BH * pb, :],
                        osb[:].rearrange("p c i d a w b -> p c (i d a) (w b)"),
                    )

            if iters <= unroll:
                for _k in range(iters):
                    body(_k)
            else:
                loop_n = iters // unroll
                rem = iters - loop_n * unroll
                with tc.For_i(0, loop_n, 1) as _i:
                    for _k in range(unroll):
                        body(_i)
                for _k in range(rem):
                    body(_k)
    nc.finalize()
    return nc


def host_prep_v2(features: np.ndarray, masks: np.ndarray):
    """Pack per-core fp16 inputs (name kept for the test harness API).

    Per band t (5 bands of 5 low-res rows), per w-tile u, a packed
    [128, 456] chunk: cols 0:256 the 9x14 feature patch (partition
    k = pr*14+pc), cols 256:456 the banded masks whose column
    (dh, a, wl, b) holds mask[dy=pr-dh, dx=pc-wl] of pixel
    (h=5t+dh, w=10u+wl), subpixel (a, b); zeros elsewhere.
    Bands 0-2 are returned as "f", bands 3-4 as "b".
    """
    f_hosts, b_hosts = [], []
    padded = np.pad(features, ((0, 0), (0, 0), (R, R), (R, R)))  # [N,C,H+4,W+4]

    # index helpers for the banded scatter (shared across cores)
    dy = np.arange(KS)[:, None, None, None]        # [5,1,1,1]
    dx = np.arange(KS)[None, :, None, None]        # [1,5,1,1]
    dh = np.arange(BH)[None, None, :, None]        # [1,1,5,1]
    wl = np.arange(BW)[None, None, None, :]        # [1,1,1,10]
    kf = (dh + dy) * RW + (wl + dx)                # [5,5,5,10] patch row index
    dhb = np.broadcast_to(dh, kf.shape)
    wlb = np.broadcast_to(wl, kf.shape)

    rows = (BH * np.arange(NT)[:, None] + np.arange(RH)[None, :])  # [5,9]
    cols = (BW * np.arange(NU)[:, None] + np.arange(RW)[None, :])  # [10,14]

    for core in range(8):
        n, q = divmod(core, 4)
        h0 = HC * q
        FB = np.zeros((128, NT, NU, FBW), np.float16)
        # features: [C, 29, 104] padded window for this core
        F_core = padded[n, :, h0 : h0 + NR, :]
        P2 = F_core.transpose(1, 2, 0)  # [29, 104, C]
        # W5[t, pr, u, pc, c] -> [pr*14+pc, t, u, c]
        W5 = P2[rows[:, :, None, None], cols[None, None, :, :], :]
        FB[: RH * RW, :, :, :C] = W5.transpose(1, 3, 0, 2, 4).reshape(
            RH * RW, NT, NU, C
        )
        # masks: m7[dy, dx, h, a, w, b] for this core's rows
        m7 = masks[n].reshape(KS, KS, H, S, W, S)[:, :, h0 : h0 + HC]
        # -> [dy, dx, dh, wl, t, u, a, b]
        m8 = m7.reshape(KS, KS, NT, BH, S, NU, BW, S)
        m8 = m8.transpose(0, 1, 3, 6, 2, 5, 4, 7)
        B8 = np.zeros((128, NT, NU, BH, S, BW, S), np.float16)
        B8[kf, :, :, dhb, :, wlb, :] = m8
        FB[:, :, :, C:] = B8.reshape(128, NT, NU, NCOL)
        f_hosts.append(
            np.ascontiguousarray(FB[:, :NTF].reshape(128, NTF * NU, FBW))
        )
        b_hosts.append(
            np.ascontiguousarray(FB[:, NTF:].reshape(128, (NT - NTF) * NU, FBW))
        )
    return f_hosts, b_hosts


_NC_CACHE = {}


def _get_program(iters: int = 1):
    if iters not in _NC_CACHE:
        _NC_CACHE[iters] = build_program(iters)
    return _NC_CACHE[iters]


def kernel(features: np.ndarray, masks: np.ndarray) -> np.ndarray:
    features = np.ascontiguousarray(features, dtype=np.float32)
    masks = np.ascontiguousarray(masks, dtype=np.float32)
    f_hosts, b_hosts = host_prep_v2(features, masks)
    in_maps = [{"f": f_hosts[c], "b": b_hosts[c]} for c in range(8)]
    nc = _get_program(1)
    res = run_bass_kernel_spmd(nc, in_maps, list(range(8)))
    out = np.empty((N, C, S * H, S * W), np.float32)
    for core in range(8):
        n, q = divmod(core, 4)
        out[n, :, S * HC * q : S * HC * (q + 1), :] = (
            res.results[core]["out"].astype(np.float32)
        )
    return out
